# revision 23
# baseline (speedup 1.0000x reference)
"""MinGRU (parallel log-space scan) Trainium2 Bass kernel.

Problem (hardcoded):
    x:    [B=8, S=4096, D=1024] f32
    W_hg: [D=1024, 2*D=2048]    f32
    out:  [B=8, S=4096, D=1024] f32

    hg = x @ W_hg ; hidden, gate = split(hg)
    h_t = (1-z_t) * h_{t-1} + z_t * g(hidden_t),  z = sigmoid(gate),
    g(v) = v + 0.5 if v >= 0 else sigmoid(v)  ==  max(v + 0.5, sigmoid(v))

Sharding: data-parallel over batch, one batch row per NeuronCore (8 cores),
W_hg replicated.

Layout strategy: the scan must run along the free dimension (channels on
partitions), so the device works entirely in the transposed layout
hg^T/h^T = [channels, seq]. The host packs x per batch row into
per-chunk-contiguous bf16 blocks and W into per-k-slice-contiguous bf16
blocks so every SBUF load is a single DMA instruction (the Sync engine
serializes DMA issues at ~0.6us each, which otherwise dominates the
kernel head).

bf16 matmuls: 1 cyc/row on the PE like fp32r, but FWL (fast weight load)
hides the LDWEIGHTS stream behind the matmuls, and the x/W DMA volume
halves. Accuracy: ~2.3e-3 max rel err, far below the 2e-2 gate.

Per-core pipeline over seq chunks of C=512:
  one DMA for the x^T chunk block [128, 8j x C] (bf16)
  -> per k: bf16 matmuls gate then hidden, accumulated in PSUM
     (a = sigmoid(-gate) on ACT overlaps the hidden matmuls)
  -> DVE: gh = (hidden + 0.5) max sigh ; bneg = (a - 1) * gh
  -> DVE: h = scan(a * h_prev) - bneg   (carry chained across chunks)
  -> DMA h^T tile straight to DRAM out^T.

Tail: the last k-tile's hidden accumulation is split in half and its
pointwise/scan/store runs in quarters so the final stores overlap the
final scans.
"""

import numpy as np

import concourse.bacc as bacc
import concourse.tile as tile
from concourse import mybir

B, S, D = 8, 4096, 1024
N_CORES = 8
P = 128  # partitions
# Seq chunk schedule: uniform 512 (the PSUM-bank maximum). Smaller lead-in
# chunks were tried and lose: the extra matmul instructions and pipeline
# gaps cost more than the smaller critical head DMA saves.
CHUNKS = [512] * 8
CHUNK_OFF = [sum(CHUNKS[:i]) for i in range(len(CHUNKS))]
assert sum(CHUNKS) == S
N_DT = D // P  # 8 d-tiles (contraction)
N_KT = D // P  # 8 output channel tiles (hidden dim = D)
WBLK = N_DT * 2 * P  # packed w k-slice columns (j-major, hidden+gate)

F32 = mybir.dt.float32
BF16 = mybir.dt.bfloat16
MM_DT = BF16

_COMPILED = {}


def _build():
    nc = bacc.Bacc(
        "TRN2", target_bir_lowering=False, debug=False, num_devices=N_CORES
    )
    # packed layouts (see make_in_maps): one contiguous run per SBUF load
    xt_d = nc.dram_tensor(
        "xt", [P, N_DT * S], MM_DT, kind="ExternalInput"
    ).ap()
    w_d = nc.dram_tensor(
        "w", [P, N_KT * WBLK], MM_DT, kind="ExternalInput"
    ).ap()
    out_d = nc.dram_tensor("outT", [D, S], F32, kind="ExternalOutput").ap()

    AL = mybir.AluOpType
    SIG = mybir.ActivationFunctionType.Sigmoid

    with tile.TileContext(nc) as tc:
        with (
            tc.tile_pool(name="wpool", bufs=1) as wpool,
            tc.tile_pool(name="xtp", bufs=3) as xt_pool,
            tc.tile_pool(name="pw", bufs=3) as pw_pool,
            tc.tile_pool(name="hp", bufs=3) as h_pool,
            tc.tile_pool(name="pshg", bufs=8, space="PSUM") as psum_hg,
        ):
            w_tile = wpool.tile([P, N_KT * WBLK], MM_DT, name="w_tile")

            def wload(k):
                nc.sync.dma_start(
                    w_tile[:, k * WBLK : (k + 1) * WBLK],
                    w_d[:, k * WBLK : (k + 1) * WBLK],
                )

            def load_x_chunk(sc, name):
                csz = CHUNKS[sc]
                off = N_DT * CHUNK_OFF[sc]
                t = xt_pool.tile([P, N_DT * csz], MM_DT, tag="xc", name=name)
                nc.sync.dma_start(t[:], xt_d[:, off : off + N_DT * csz])
                return t

            # PE p-state warmup: the tensor engine ramps 0.65->2.4 GHz over
            # ~3us of continuous execution. Run garbage matmuls on memset
            # tiles while the first real DMAs are in flight so the ramp
            # cost overlaps the head instead of the real stream.
            warm_w = xt_pool.tile([P, P], MM_DT, tag="warm_w", bufs=1)
            warm_x = xt_pool.tile([P, 512], MM_DT, tag="warm_x", bufs=1)
            nc.vector.memset(warm_w[:], 0.0)
            nc.vector.memset(warm_x[:], 0.0)
            warm_ps = psum_hg.tile([P, 512], F32, tag="ph")
            for i in range(14):
                nc.tensor.matmul(
                    warm_ps[:], warm_w[:], warm_x[:],
                    start=(i == 0), stop=(i == 13),
                )

            # Critical path first: chunk 0 of x^T (the largest transfer)
            # then the k=0 weight slice — every load a single DMA issue
            # (the Sync engine serializes issues at ~0.65us each).
            x0 = load_x_chunk(0, "x0")
            wload(0)
            # Hold back the non-critical loads until x0 has landed so they
            # don't steal DMA bandwidth from it: this 4-byte DMA reads x0,
            # so the in-order Sync engine blocks here until x0 completes.
            fence = xt_pool.tile([P, 2], MM_DT, tag="fence", bufs=1)
            nc.sync.dma_start(fence[0:1, 0:2], x0[0:1, 0:2])
            wload(1)
            x1 = load_x_chunk(1, "x1")
            for k in range(2, N_KT):
                wload(k)

            # lhsT slices: w_sb[kk][j]; kk in [0,8) hidden, [8,16) gate
            w_sb = [
                [
                    w_tile[
                        :,
                        k * WBLK + j * 2 * P + b * P :
                        k * WBLK + j * 2 * P + (b + 1) * P,
                    ]
                    for j in range(N_DT)
                ]
                for b in range(2)
                for k in range(N_KT)
            ]

            prev_h = [None] * N_KT
            for sc, csz in enumerate(CHUNKS):
                s0 = CHUNK_OFF[sc]
                last_chunk = sc == len(CHUNKS) - 1
                if sc == 0:
                    xts = x0
                elif sc == 1:
                    xts = x1
                else:
                    xts = load_x_chunk(sc, None)

                def mm_group(ps, kk, lo, hi):
                    for j in range(N_DT):
                        nc.tensor.matmul(
                            ps[:],
                            w_sb[kk][j],
                            xts[:, j * csz + lo : j * csz + hi],
                            start=(j == 0),
                            stop=(j == N_DT - 1),
                        )

                for k in range(N_KT):
                    last_k = last_chunk and k == N_KT - 1
                    # gate first: a = sigmoid(-gate) is ready while the
                    # hidden matmuls run, shortening the per-k tail chain
                    pg = psum_hg.tile([P, csz], F32, tag="ph")  # gate
                    mm_group(pg, N_KT + k, 0, csz)
                    a_t = pw_pool.tile([P, csz], F32, tag="a")
                    nc.scalar.activation(a_t[:], pg[:], SIG, scale=-1.0)
                    if last_k:
                        # split the last accumulation (separate PSUM banks:
                        # a start flag zeroes the whole 2KB zero-region) so
                        # the pointwise tail starts before the final matmul
                        hh = csz // 2
                        ph_a = psum_hg.tile([P, hh], F32, tag="ph")
                        ph_b = psum_hg.tile([P, hh], F32, tag="ph")
                        for j in range(N_DT):
                            nc.tensor.matmul(
                                ph_a[:], w_sb[k][j],
                                xts[:, j * csz : j * csz + hh],
                                start=(j == 0), stop=(j == N_DT - 1),
                            )
                        for j in range(N_DT):
                            nc.tensor.matmul(
                                ph_b[:], w_sb[k][j],
                                xts[:, j * csz + hh : (j + 1) * csz],
                                start=(j == 0), stop=(j == N_DT - 1),
                            )

                        def ph_piece(lo, hi):
                            if hi <= hh:
                                return ph_a[:, lo:hi]
                            assert lo >= hh
                            return ph_b[:, lo - hh : hi - hh]
                    else:
                        ph = psum_hg.tile([P, csz], F32, tag="ph")  # hidden
                        mm_group(ph, k, 0, csz)

                        def ph_piece(lo, hi):
                            return ph[:, lo:hi]

                    # pointwise/scan split: halves for the very last k-tile
                    # (matches the hidden PSUM halves; finer splits lose to
                    # per-instruction overhead since all three pointwise ops
                    # serialize on the DVE)
                    nsplit = 2 if last_k else 1
                    sigh = pw_pool.tile([P, csz], F32, tag="sigh")
                    gh = pw_pool.tile([P, csz], F32, tag="gh")
                    bneg = pw_pool.tile([P, csz], F32, tag="bneg")
                    h = h_pool.tile([P, csz], F32, tag=f"h{k}")
                    H = csz // nsplit
                    for q in range(nsplit):
                        lo, hi = q * H, (q + 1) * H
                        php = ph_piece(lo, hi)
                        # sigh = sigmoid(hidden)
                        nc.scalar.activation(sigh[:, lo:hi], php, SIG)
                        # g(hidden) = max(hidden + 0.5, sigmoid(hidden))
                        nc.vector.scalar_tensor_tensor(
                            gh[:, lo:hi], php, 0.5, sigh[:, lo:hi],
                            op0=AL.add, op1=AL.max,
                        )
                        # bneg = (a - 1) * g = -(z * g)
                        nc.vector.scalar_tensor_tensor(
                            bneg[:, lo:hi], a_t[:, lo:hi], 1.0, gh[:, lo:hi],
                            op0=AL.subtract, op1=AL.mult,
                        )
                        # h_t = a_t * h_{t-1} - bneg_t  (linear recurrence)
                        if q == 0:
                            init = (
                                0.0
                                if prev_h[k] is None
                                else prev_h[k][:, -1:]
                            )
                        else:
                            init = h[:, lo - 1 : lo]
                        nc.vector.tensor_tensor_scan(
                            h[:, lo:hi], a_t[:, lo:hi], bneg[:, lo:hi], init,
                            op0=AL.mult, op1=AL.subtract,
                        )
                        nc.sync.dma_start(
                            out_d[k * P : (k + 1) * P, s0 + lo : s0 + hi],
                            h[:, lo:hi],
                        )
                    prev_h[k] = h
    nc.compile()
    return nc


def _get_nc():
    key = str(MM_DT)
    if key not in _COMPILED:
        _COMPILED[key] = _build()
    return _COMPILED[key]


def make_in_maps(x: np.ndarray, W_hg: np.ndarray) -> list[dict]:
    import ml_dtypes

    bf = ml_dtypes.bfloat16
    x = np.asarray(x, dtype=np.float32)
    w = np.asarray(W_hg, dtype=np.float32)

    # x pack: per-chunk contiguous blocks [p, sc][j, t]
    def pack_x(xb):
        blocks = []
        for sc, csz in enumerate(CHUNKS):
            s0 = CHUNK_OFF[sc]
            blk = xb[s0 : s0 + csz, :]  # [csz, D]
            blocks.append(
                blk.reshape(csz, N_DT, P).transpose(2, 1, 0).reshape(P, -1)
            )
        return np.ascontiguousarray(np.concatenate(blocks, axis=1).astype(bf))

    xp = [pack_x(x[b]) for b in range(N_CORES)]
    # w pack: W[j*128+p, b*1024 + k*128 + c] -> wp[p, k, j, b, c]
    wp = np.ascontiguousarray(
        w.reshape(N_DT, P, 2, N_KT, P)
        .transpose(1, 3, 0, 2, 4)
        .reshape(P, N_KT * WBLK)
        .astype(bf)
    )
    return [{"xt": xp[b], "w": wp} for b in range(N_CORES)]


def kernel(x: np.ndarray, W_hg: np.ndarray) -> np.ndarray:
    from concourse.bass_utils import run_bass_kernel_spmd

    assert x.shape == (B, S, D) and W_hg.shape == (D, 2 * D)
    nc = _get_nc()
    in_maps = make_in_maps(x, W_hg)
    res = run_bass_kernel_spmd(nc, in_maps, list(range(N_CORES)))
    out = np.empty((B, S, D), dtype=np.float32)
    for b in range(N_CORES):
        out[b] = res.results[b]["outT"].T
    return out


# revision 25
# speedup vs baseline: 1.0114x; 1.0114x over previous
"""MinGRU (parallel log-space scan) Trainium2 Bass kernel.

Problem (hardcoded):
    x:    [B=8, S=4096, D=1024] f32
    W_hg: [D=1024, 2*D=2048]    f32
    out:  [B=8, S=4096, D=1024] f32

    hg = x @ W_hg ; hidden, gate = split(hg)
    h_t = (1-z_t) * h_{t-1} + z_t * g(hidden_t),  z = sigmoid(gate),
    g(v) = v + 0.5 if v >= 0 else sigmoid(v)  ==  max(v + 0.5, sigmoid(v))

Sharding: data-parallel over batch, one batch row per NeuronCore (8 cores),
W_hg replicated.

Layout strategy: the scan must run along the free dimension (channels on
partitions), so the device works entirely in the transposed layout
hg^T/h^T = [channels, seq]. The host packs x per batch row into
per-chunk-contiguous bf16 blocks and W into per-k-slice-contiguous bf16
blocks so every SBUF load is a single DMA instruction (the Sync engine
serializes DMA issues at ~0.6us each, which otherwise dominates the
kernel head).

bf16 matmuls: 1 cyc/row on the PE like fp32r, but FWL (fast weight load)
hides the LDWEIGHTS stream behind the matmuls, and the x/W DMA volume
halves. Accuracy: ~2.3e-3 max rel err, far below the 2e-2 gate.

Per-core pipeline over seq chunks of C=512:
  one DMA for the x^T chunk block [128, 8j x C] (bf16)
  -> per k: bf16 matmuls gate then hidden, accumulated in PSUM
     (a = sigmoid(-gate) on ACT overlaps the hidden matmuls)
  -> DVE: gh = (hidden + 0.5) max sigh ; bneg = (a - 1) * gh
  -> DVE: h = scan(a * h_prev) - bneg   (carry chained across chunks)
  -> DMA h^T tile straight to DRAM out^T.

Head: PE p-state warmup matmuls on memset tiles bridge the ~6us wait for
the first DMAs so the 0.65->2.4 GHz clock ramp overlaps the head, and a
4-byte fence DMA holds the non-critical loads back until chunk 0 has
landed so it gets the full DMA bandwidth.

Tail: the last k-tile's hidden accumulation is split in half and its
pointwise/scan/store runs in halves so the final stores overlap the
final scans.
"""

import numpy as np

import concourse.bacc as bacc
import concourse.tile as tile
from concourse import mybir

B, S, D = 8, 4096, 1024
N_CORES = 8
P = 128  # partitions
# Seq chunk schedule: uniform 512 (the PSUM-bank maximum). Smaller lead-in
# chunks were tried and lose: the extra matmul instructions and pipeline
# gaps cost more than the smaller critical head DMA saves.
CHUNKS = [512] * 8
CHUNK_OFF = [sum(CHUNKS[:i]) for i in range(len(CHUNKS))]
assert sum(CHUNKS) == S
N_DT = D // P  # 8 d-tiles (contraction)
N_KT = D // P  # 8 output channel tiles (hidden dim = D)
WBLK = N_DT * 2 * P  # packed w k-slice columns (j-major, hidden+gate)

F32 = mybir.dt.float32
BF16 = mybir.dt.bfloat16
MM_DT = BF16

_COMPILED = {}


def _build():
    nc = bacc.Bacc(
        "TRN2", target_bir_lowering=False, debug=False, num_devices=N_CORES
    )
    # packed layouts (see make_in_maps): one contiguous run per SBUF load
    xt_d = nc.dram_tensor(
        "xt", [P, N_DT * S], MM_DT, kind="ExternalInput"
    ).ap()
    w_d = nc.dram_tensor(
        "w", [P, N_KT * WBLK], MM_DT, kind="ExternalInput"
    ).ap()
    out_d = nc.dram_tensor("outT", [D, S], F32, kind="ExternalOutput").ap()

    AL = mybir.AluOpType
    SIG = mybir.ActivationFunctionType.Sigmoid

    with tile.TileContext(nc) as tc:
        with (
            tc.tile_pool(name="wpool", bufs=1) as wpool,
            tc.tile_pool(name="xtp", bufs=3) as xt_pool,
            tc.tile_pool(name="pw", bufs=3) as pw_pool,
            tc.tile_pool(name="hp", bufs=3) as h_pool,
            tc.tile_pool(name="pshg", bufs=8, space="PSUM") as psum_hg,
        ):
            w_tile = wpool.tile([P, N_KT * WBLK], MM_DT, name="w_tile")

            def wload(k):
                nc.sync.dma_start(
                    w_tile[:, k * WBLK : (k + 1) * WBLK],
                    w_d[:, k * WBLK : (k + 1) * WBLK],
                )

            def load_x_chunk(sc, name):
                csz = CHUNKS[sc]
                off = N_DT * CHUNK_OFF[sc]
                t = xt_pool.tile([P, N_DT * csz], MM_DT, tag="xc", name=name)
                nc.sync.dma_start(t[:], xt_d[:, off : off + N_DT * csz])
                return t

            # PE p-state warmup: the tensor engine ramps 0.65->2.4 GHz over
            # ~3us of continuous execution. Run garbage matmuls on memset
            # tiles while the first real DMAs are in flight so the ramp
            # cost overlaps the head instead of the real stream.
            warm_w = xt_pool.tile([P, P], MM_DT, tag="warm_w", bufs=1)
            warm_x = xt_pool.tile([P, 512], MM_DT, tag="warm_x", bufs=1)
            nc.vector.memset(warm_w[:], 0.0)
            nc.vector.memset(warm_x[:], 0.0)
            warm_ps = psum_hg.tile([P, 512], F32, tag="ph")
            for i in range(14):
                nc.tensor.matmul(
                    warm_ps[:], warm_w[:], warm_x[:],
                    start=(i == 0), stop=(i == 13),
                )

            # Critical path first: chunk 0 of x^T (the largest transfer)
            # then the k=0 weight slice — every load a single DMA issue
            # (the Sync engine serializes issues at ~0.65us each).
            x0 = load_x_chunk(0, "x0")
            wload(0)
            # Hold back the non-critical loads until x0 has landed so they
            # don't steal DMA bandwidth from it: this 4-byte DMA reads x0,
            # so the in-order Sync engine blocks here until x0 completes.
            fence = xt_pool.tile([P, 2], MM_DT, tag="fence", bufs=1)
            nc.sync.dma_start(fence[0:1, 0:2], x0[0:1, 0:2])
            wload(1)
            x1 = load_x_chunk(1, "x1")
            for k in range(2, N_KT):
                wload(k)

            # lhsT slices: w_sb[kk][j]; kk in [0,8) hidden, [8,16) gate
            w_sb = [
                [
                    w_tile[
                        :,
                        k * WBLK + j * 2 * P + b * P :
                        k * WBLK + j * 2 * P + (b + 1) * P,
                    ]
                    for j in range(N_DT)
                ]
                for b in range(2)
                for k in range(N_KT)
            ]

            prev_h = [None] * N_KT
            for sc, csz in enumerate(CHUNKS):
                s0 = CHUNK_OFF[sc]
                last_chunk = sc == len(CHUNKS) - 1
                if sc == 0:
                    xts = x0
                elif sc == 1:
                    xts = x1
                else:
                    xts = load_x_chunk(sc, None)

                def mm_group(ps, kk, lo, hi):
                    for j in range(N_DT):
                        nc.tensor.matmul(
                            ps[:],
                            w_sb[kk][j],
                            xts[:, j * csz + lo : j * csz + hi],
                            start=(j == 0),
                            stop=(j == N_DT - 1),
                        )

                for k in range(N_KT):
                    last_k = last_chunk and k == N_KT - 1
                    # gate first: a = sigmoid(-gate) is ready while the
                    # hidden matmuls run, shortening the per-k tail chain
                    pg = psum_hg.tile([P, csz], F32, tag="ph")  # gate
                    mm_group(pg, N_KT + k, 0, csz)
                    a_t = pw_pool.tile([P, csz], F32, tag="a")
                    nc.scalar.activation(a_t[:], pg[:], SIG, scale=-1.0)
                    if last_k:
                        # split the last accumulation into quarter groups on
                        # separate PSUM banks (a start flag zeroes the whole
                        # 2KB zero-region) aligned with the pointwise
                        # quarters below, so 3 of 4 quarters' DVE work
                        # overlaps the remaining matmuls and only the final
                        # quarter's chain is exposed after the last matmul
                        qw = csz // 4
                        ph_q = []
                        for q in range(4):
                            pq = psum_hg.tile([P, qw], F32, tag="ph")
                            for j in range(N_DT):
                                nc.tensor.matmul(
                                    pq[:], w_sb[k][j],
                                    xts[:, j * csz + q * qw :
                                        j * csz + (q + 1) * qw],
                                    start=(j == 0), stop=(j == N_DT - 1),
                                )
                            ph_q.append(pq)

                        def ph_piece(lo, hi):
                            q, r = divmod(lo, qw)
                            assert hi - lo == qw and r == 0
                            return ph_q[q][:]
                    else:
                        ph = psum_hg.tile([P, csz], F32, tag="ph")  # hidden
                        mm_group(ph, k, 0, csz)

                        def ph_piece(lo, hi):
                            return ph[:, lo:hi]

                    # pointwise/scan split: quarters for the very last
                    # k-tile (aligned with its hidden PSUM quarter groups)
                    nsplit = 4 if last_k else 1
                    sigh = pw_pool.tile([P, csz], F32, tag="sigh")
                    gh = pw_pool.tile([P, csz], F32, tag="gh")
                    bneg = pw_pool.tile([P, csz], F32, tag="bneg")
                    h = h_pool.tile([P, csz], F32, tag=f"h{k}")
                    H = csz // nsplit
                    for q in range(nsplit):
                        lo, hi = q * H, (q + 1) * H
                        php = ph_piece(lo, hi)
                        # sigh = sigmoid(hidden)
                        nc.scalar.activation(sigh[:, lo:hi], php, SIG)
                        # g(hidden) = max(hidden + 0.5, sigmoid(hidden))
                        nc.vector.scalar_tensor_tensor(
                            gh[:, lo:hi], php, 0.5, sigh[:, lo:hi],
                            op0=AL.add, op1=AL.max,
                        )
                        # bneg = (a - 1) * g = -(z * g)
                        nc.vector.scalar_tensor_tensor(
                            bneg[:, lo:hi], a_t[:, lo:hi], 1.0, gh[:, lo:hi],
                            op0=AL.subtract, op1=AL.mult,
                        )
                        # h_t = a_t * h_{t-1} - bneg_t  (linear recurrence)
                        if q == 0:
                            init = (
                                0.0
                                if prev_h[k] is None
                                else prev_h[k][:, -1:]
                            )
                        else:
                            init = h[:, lo - 1 : lo]
                        nc.vector.tensor_tensor_scan(
                            h[:, lo:hi], a_t[:, lo:hi], bneg[:, lo:hi], init,
                            op0=AL.mult, op1=AL.subtract,
                        )
                        nc.sync.dma_start(
                            out_d[k * P : (k + 1) * P, s0 + lo : s0 + hi],
                            h[:, lo:hi],
                        )
                    prev_h[k] = h
    nc.compile()
    return nc


def _get_nc():
    key = str(MM_DT)
    if key not in _COMPILED:
        _COMPILED[key] = _build()
    return _COMPILED[key]


def make_in_maps(x: np.ndarray, W_hg: np.ndarray) -> list[dict]:
    import ml_dtypes

    bf = ml_dtypes.bfloat16
    x = np.asarray(x, dtype=np.float32)
    w = np.asarray(W_hg, dtype=np.float32)

    # x pack: per-chunk contiguous blocks [p, sc][j, t]
    def pack_x(xb):
        blocks = []
        for sc, csz in enumerate(CHUNKS):
            s0 = CHUNK_OFF[sc]
            blk = xb[s0 : s0 + csz, :]  # [csz, D]
            blocks.append(
                blk.reshape(csz, N_DT, P).transpose(2, 1, 0).reshape(P, -1)
            )
        return np.ascontiguousarray(np.concatenate(blocks, axis=1).astype(bf))

    xp = [pack_x(x[b]) for b in range(N_CORES)]
    # w pack: W[j*128+p, b*1024 + k*128 + c] -> wp[p, k, j, b, c]
    wp = np.ascontiguousarray(
        w.reshape(N_DT, P, 2, N_KT, P)
        .transpose(1, 3, 0, 2, 4)
        .reshape(P, N_KT * WBLK)
        .astype(bf)
    )
    return [{"xt": xp[b], "w": wp} for b in range(N_CORES)]


def kernel(x: np.ndarray, W_hg: np.ndarray) -> np.ndarray:
    from concourse.bass_utils import run_bass_kernel_spmd

    assert x.shape == (B, S, D) and W_hg.shape == (D, 2 * D)
    nc = _get_nc()
    in_maps = make_in_maps(x, W_hg)
    res = run_bass_kernel_spmd(nc, in_maps, list(range(N_CORES)))
    out = np.empty((B, S, D), dtype=np.float32)
    for b in range(N_CORES):
        out[b] = res.results[b]["outT"].T
    return out
